# revision 52
# baseline (speedup 1.0000x reference)
"""Causal self-attention Trainium2 kernel.

Reference computation (B=2, T=2048, EMB=1024, H=16 heads, D=64):
    qkv = x @ Wqkv + bqkv ; split q,k,v ; per-head causal softmax attention ;
    out = concat_heads @ Wout + bout

Sharding: 8 cores = data-parallel over batch (2) x tensor-parallel over
heads (4 heads/core).  Each core computes, for its (batch b, head shard m):
  - qkT = (x_b @ Wqk_m)^T  in d-major layout (+ bias); k is stored
    zero-padded per head in kz so score matmuls contract over the full 128
    partitions (K=64 matmuls run ~2.7x slower per row on TRN2, and
    tile_position row-tiling does not recover the loss -- measured)
  - v   = x_b @ Wv_m       in t-major layout (v bias folded into bout on
    host), t-tile pairs share one PSUM bank and one merged DVE copy.  The k
    bias-add lands once in a packed kT tile; the per-head zero-padded kz
    blocks are scattered from it by SBUF->SBUF DMA on the idle DMA engines
  - per head pair (a=0,1): S^T = k q^T, E = exp(S^T) restricted to the
    causal region (one tri-mask multiply covers both heads per diagonal
    block), O'^T = [v | 1]^T E^T (ones column yields softmax sums),
    normalize via reciprocal_approx_fast on an SBUF-staged sums row (the
    microcoded InstReciprocal costs ~2.5us/instance on HW) + gpsimd
    partition_broadcast
  - partial out = O_norm^T^T @ Wout_m  -> host sums the 4 TP partials per
    batch.  Out-proj chunks are deferred past the group boundary so they
    never stall the PE on the just-emitted norm chain.
All host-side inputs are pre-permuted so every DMA moves >=2KB contiguous
per partition (128 descriptors per transfer).  All matmul operands are bf16
with fp32 PSUM accumulation; ~3e-3 scale-relative error.
Measured per-iteration device time (repeat-delta, matched measurement
windows): 160 us vs 206 us for the previous-session kernel in the same
container (fast-window readings reached 148 us).
"""

import sys

sys.path.insert(0, "/opt/trn_rl_repo")

import numpy as np

B, T, EMB = 2, 2048, 1024
H, D = 16, 64
N_CORES = 8
TP = 4  # head shards
HEADS_PER_CORE = H // TP  # 4
FSH = HEADS_PER_CORE * D  # 256 features per shard for each of q,k,v
P = 128
NG = T // 512  # 4 query groups of 512
NT = T // P  # 16 tiles of 128

_prog_cache = {}


def _build_program(repeat=1, full_repeat=False, ablate=None):
    ablate = ablate or set()
    import concourse.mybir as mybir
    import concourse.tile as tile
    from concourse import bacc

    f32 = mybir.dt.float32
    bf16 = mybir.dt.bfloat16
    AF = mybir.ActivationFunctionType
    OP = mybir.AluOpType

    nc = bacc.Bacc("TRN2", target_bir_lowering=False)

    # host-side pre-permuted layouts: partition-major, contiguous per partition
    xT_d = nc.dram_tensor("xT", (P, NG * EK_ * 512), bf16, kind="ExternalInput")
    wqk_d = nc.dram_tensor("wqk", (P, 4 * EK_ * P), bf16, kind="ExternalInput")
    wv_d = nc.dram_tensor("wv", (P, EK_ * FSH), bf16, kind="ExternalInput")
    wout_d = nc.dram_tensor("wout", (P, 2 * EMB), bf16, kind="ExternalInput")
    bqk_d = nc.dram_tensor("bqk", (P, 4), f32, kind="ExternalInput")
    tri_d = nc.dram_tensor("trimask", (P, P), bf16, kind="ExternalInput")
    out_d = nc.dram_tensor("out", (P, (NT // 2) * 2 * EMB), bf16, kind="ExternalOutput")

    EK = EK_  # 8 contraction chunks
    xT_r = xT_d.rearrange("p (g e t) -> p g e t", g=NG, e=EK)
    wqk_r = wqk_d.rearrange("p (f e c) -> p f e c", f=4, e=EK)
    out_r = out_d.rearrange("p (i two_e) -> p i two_e", i=NT // 2)

    with nc.allow_low_precision(
        reason="bf16 matmul operands; fp32 accumulation in PSUM"
    ), tile.TileContext(nc) as tc:
        with (
            tc.tile_pool(name="consts", bufs=1) as consts,
            tc.tile_pool(name="qkt", bufs=1) as qkt_p,
            tc.tile_pool(name="vp", bufs=1) as v_p,
            tc.tile_pool(name="onorm", bufs=1) as onorm_p,
            tc.tile_pool(name="xp", bufs=2) as xp,
            tc.tile_pool(name="ep", bufs=3) as e_p,
            tc.tile_pool(name="bc", bufs=6) as bc_p,
            tc.tile_pool(name="outsb", bufs=3) as out_p,
            tc.tile_pool(name="psP", bufs=2, space="PSUM") as psP,
            tc.tile_pool(name="psS", bufs=1, space="PSUM") as psS,
            tc.tile_pool(name="psO", bufs=1, space="PSUM") as psO,
        ):
            # Startup-critical ordering: bias (tiny) + wqk f-tile 0 first, then
            # x chunk 0 is issued in the rep loop, then remaining weights are
            # staged so each arrives just before its first consumer.
            wqk_fs = [
                consts.tile([P, EK, P], bf16, name=f"wqk_f{f}") for f in range(4)
            ]
            wv_sb = consts.tile([P, EK, FSH], bf16)
            bqk_sb = consts.tile([P, 4], f32)
            tri_sb = consts.tile([P, P], bf16)
            wout_sb = consts.tile([P, 2, EMB], bf16)

            nc.sync.dma_start(bqk_sb[:], bqk_d[:])
            # halves so the first qk matmul (contracting e=0..7 in order)
            # can start after half the bytes land (subtile deps)
            nc.sync.dma_start(wqk_fs[0][:, 0:4], wqk_r[:, 0, 0:4])
            nc.sync.dma_start(wqk_fs[0][:, 4:8], wqk_r[:, 0, 4:8])

            ones_f32 = consts.tile([P, 1], f32)
            nc.vector.memset(ones_f32[:], 1.0)
            # preload the Exp activation table set during startup (the
            # implicit ACT_TABLE_LOAD otherwise stalls the first real exp
            # ~2.7us mid-kernel)
            act_warm = consts.tile([1, 1], bf16, name="act_warm")
            nc.scalar.activation(act_warm[:], ones_f32[0:1, 0:1], AF.Exp)
            ones_row = consts.tile([1, D], bf16, name="ones_row")
            nc.vector.tensor_copy(
                ones_row[:], ones_f32[0:1, 0:1].to_broadcast((1, D))
            )
            scratch_sb = None
            if "dve_double" in ablate or "dve_sbuf_double" in ablate:
                scratch_sb = consts.tile([P, EMB], bf16, name="scratch")
            COPIES_OFF = "copies_off" in ablate
            bc_fixed = None
            if "no_bcast" in ablate:
                bc_fixed = consts.tile([D, 512], f32, name="bc_fixed")
                nc.vector.memset(bc_fixed[:], 0.01)

            def load_mid_consts():
                nc.sync.dma_start(wqk_fs[1][:], wqk_r[:, 1])
                nc.sync.dma_start(wv_sb[:], wv_d[:])
                nc.sync.dma_start(wqk_fs[2][:], wqk_r[:, 2])
                nc.sync.dma_start(wqk_fs[3][:], wqk_r[:, 3])
                nc.sync.dma_start(tri_sb[:], tri_d[:])

            def load_wout():
                nc.sync.dma_start(wout_sb[:], wout_d[:])

            # qkT[:, f, t]: f 0..1 = q features; head h=2hp+a: q at
            # qkT[64a:64a+64, hp, :].  k is stored zero-padded per head in kz
            # so score matmuls contract over the full 128 partitions (K=64
            # matmuls run ~2.7x slower per row on TRN2 hardware, and
            # tile_position row-tiling does not recover the loss).
            qkT = qkt_p.tile([P, 4, T], bf16)
            kT = qkt_p.tile([P, 2, T], bf16, name="kT")
            kz = qkt_p.tile([P, 2, 2, T], bf16, name="kz")
            nc.vector.memset(kz[0:64, :, 1, :], 0.0)
            nc.vector.memset(kz[64:128, :, 0, :], 0.0)
            v_sb = v_p.tile([P, NT, HEADS_PER_CORE * (D + 1)], bf16)
            v_ones_view = v_sb.rearrange("p t (h c) -> p t h c", c=D + 1)[:, :, :, D]
            if COPIES_OFF or "qkvcopies_off" in ablate:
                nc.vector.memset(v_sb[:], 1.0)
                nc.vector.memset(qkT[:], 0.01)
            nc.vector.tensor_copy(
                v_ones_view, ones_f32[:, 0:1].to_broadcast((P, NT, HEADS_PER_CORE))
            )

            for _rep in range(repeat):
                if full_repeat and _rep > 0:
                    # re-issue every per-call fixed cost so the repeat slope
                    # measures the FULL single-shot program time
                    nc.sync.dma_start(bqk_sb[:], bqk_d[:])
                    nc.sync.dma_start(wqk_fs[0][:, 0:4], wqk_r[:, 0, 0:4])
                    nc.sync.dma_start(wqk_fs[0][:, 4:8], wqk_r[:, 0, 4:8])
                    nc.vector.memset(kz[0:64, :, 1, :], 0.0)
                    nc.vector.memset(kz[64:128, :, 0, :], 0.0)
                    nc.vector.tensor_copy(
                        v_ones_view,
                        ones_f32[:, 0:1].to_broadcast((P, NT, HEADS_PER_CORE)),
                    )
                onorm = onorm_p.tile([P, 2, T], bf16)
                if COPIES_OFF or "norm_off" in ablate:
                    nc.vector.memset(onorm[:, :, 0:1], 0.5)

                # ---------- chunked x load + deferred PE work ----------
                x_chunks = {}

                def load_x_chunk(g):
                    x_chunks[g] = xp.tile([P, EK, 512], bf16, name="xchunk")
                    if "no_xdma" not in ablate:
                        if g == 0:
                            nc.sync.dma_start(
                                x_chunks[g][:, 0:4], xT_r[:, g, 0:4]
                            )
                            nc.sync.dma_start(
                                x_chunks[g][:, 4:8], xT_r[:, g, 4:8]
                            )
                        else:
                            nc.sync.dma_start(x_chunks[g][:], xT_r[:, g])
                    else:
                        nc.sync.dma_start(
                            x_chunks[g][:, 0:1, 0:1], xT_r[:, g, 0:1, 0:1]
                        )

                def qk_chunk(f, g):
                    def run():
                        ps = psP.tile([P, 512], f32, tag="pp", name="ps_qk")
                        for e in range(EK):
                            nc.tensor.matmul(
                                ps[:],
                                wqk_fs[f][:, e, :],
                                x_chunks[g][:, e, :],
                                start=(e == 0),
                                stop=(e == EK - 1),
                            )
                        if COPIES_OFF or "qkvcopies_off" in ablate:
                            return
                        if f < 2:
                            if "bias_copy" in ablate:
                                nc.vector.tensor_copy(
                                    qkT[:, f, 512 * g : 512 * (g + 1)], ps[:]
                                )
                            else:
                                nc.vector.tensor_scalar_add(
                                    qkT[:, f, 512 * g : 512 * (g + 1)],
                                    ps[:],
                                    bqk_sb[:, f : f + 1],
                                )
                        else:
                            hp = f - 2
                            # one bias-add into the packed kT layout, then
                            # scatter the two head blocks into kz's padded
                            # layout via SBUF->SBUF DMA on the idle DMA
                            # engines (saves one DVE op per k chunk)
                            nc.vector.tensor_scalar_add(
                                kT[:, hp, 512 * g : 512 * (g + 1)],
                                ps[:],
                                bqk_sb[:, f : f + 1],
                            )
                            for aa in range(2):
                                nc.sync.dma_start(
                                    kz[
                                        64 * aa : 64 * aa + 64,
                                        hp,
                                        aa,
                                        512 * g : 512 * (g + 1),
                                    ],
                                    kT[
                                        64 * aa : 64 * aa + 64,
                                        hp,
                                        512 * g : 512 * (g + 1),
                                    ],
                                )
                    return run

                def v_chunk(tp):
                    # one chunk covers the t-tile pair (2tp, 2tp+1): same PSUM
                    # bank, single merged copy
                    def run():
                        t0 = 2 * tp
                        g = t0 // 4
                        ps = psP.tile([P, 2, FSH], f32, tag="pp", name="ps_v")
                        for tt in range(2):
                            lt = (t0 + tt) % 4
                            for e in range(EK):
                                nc.tensor.matmul(
                                    ps[:, tt, :],
                                    x_chunks[g][:, e, P * lt : P * (lt + 1)],
                                    wv_sb[:, e, :],
                                    start=(e == 0),
                                    stop=(e == EK - 1),
                                )
                        if not (COPIES_OFF or "qkvcopies_off" in ablate):
                            nc.vector.tensor_copy(
                                v_sb[:, t0 : t0 + 2].rearrange(
                                    "p t (h c) -> p t h c", c=D + 1
                                )[:, :, :, :D],
                                ps[:].rearrange(
                                    "p t (h c) -> p t h c", h=HEADS_PER_CORE
                                ),
                            )
                    return run

                def outproj_chunk(i, n):
                    def run():
                        if "no_out" in ablate:
                            return
                        po = psP.tile([P, 512], f32, tag="pp", name="ps_out")
                        for p2 in range(2):
                            nc.tensor.matmul(
                                po[:],
                                onorm[:, p2, P * i : P * (i + 1)],
                                wout_sb[:, p2, 512 * n : 512 * (n + 1)],
                                start=(p2 == 0),
                                stop=(p2 == 1),
                            )
                        ip, ih = divmod(i, 2)
                        osb = out_tiles[ip]
                        if not COPIES_OFF:
                            nc.vector.tensor_copy(
                                osb[:, ih, n * 512 : (n + 1) * 512], po[:]
                            )
                        else:
                            nc.vector.memset(osb[:, ih, n * 512 : n * 512 + 1], 0.0)
                        if "dve_sbuf_double" in ablate:
                            nc.vector.tensor_copy(
                                scratch_sb[:, n * 512 : (n + 1) * 512],
                                osb[:, ih, n * 512 : (n + 1) * 512],
                            )
                        if "dve_double" in ablate:
                            nc.vector.tensor_copy(
                                scratch_sb[:, n * 512 : (n + 1) * 512], po[:]
                            )
                        out_done[ip] = out_done.get(ip, 0) + 1
                        if out_done[ip] == 4:
                            nc.sync.dma_start(
                                out_r[:, ip],
                                osb[:].rearrange("p a b -> p (a b)"),
                            )
                    return run

                out_tiles = {}
                out_done = {}

                # group 0 prerequisites up front
                load_x_chunk(0)
                if _rep == 0 or full_repeat:
                    load_mid_consts()
                qkv_fillers = [qk_chunk(f, 0) for f in range(4)]
                qkv_fillers += [v_chunk(tp) for tp in range(2)]
                defer_fillers = []

                def pop_filler():
                    # qkv chunks first (deps always ready); deferred out-proj
                    # chunks later so the group's norm chain has finished
                    if qkv_fillers:
                        qkv_fillers.pop(0)()
                    elif defer_fillers:
                        defer_fillers.pop(0)()

                for g in range(NG):
                    # qkv chunks for attention(g) must be emitted now; the
                    # deferred out-proj chunks of g-1 are NOT flushed here --
                    # they would stall the PE on the just-emitted norm chain
                    for fn_ in qkv_fillers:
                        fn_()
                    qkv_fillers = []
                    if g + 1 < NG:
                        load_x_chunk(g + 1)
                        if g == 0 and (_rep == 0 or full_repeat):
                            load_wout()
                        qkv_fillers += [qk_chunk(f, g + 1) for f in range(4)]
                        qkv_fillers += [
                            v_chunk(tp) for tp in range(2 * (g + 1), 2 * (g + 2))
                        ]

                    njt = 4 * g + 4
                    nbatch = njt // 2
                    for hp in range(2):
                        o_ps = {}
                        for a in range(2):
                            o_ps[a] = psO.tile(
                                [D + 1, 512], f32, tag=f"o{a}", name=f"o_ps{a}"
                            )
                        for u in range(nbatch):
                            s_ps = {}
                            for a in range(2):
                                s_ps[a] = psS.tile(
                                    [P, 2, 512], f32, tag=f"s{a}", name=f"s_ps{a}"
                                )
                            for jj in range(2):
                                jt = 2 * u + jj
                                cs0 = P * (jt - 4 * g) if jt > 4 * g else 0
                                for a in range(2):
                                    nc.tensor.matmul(
                                        s_ps[a][:, jj, cs0:512],
                                        kz[:, hp, a, P * jt : P * (jt + 1)],
                                        qkT[
                                            :,
                                            hp,
                                            512 * g + cs0 : 512 * (g + 1),
                                        ],
                                        start=True,
                                        stop=True,
                                    )
                            # emit one deferred chunk here so the PE has
                            # independent work queued between the score
                            # matmuls and the exp-dependent AV matmuls
                            pop_filler()
                            e_t = e_p.tile(
                                [P, 2, 2, 512], bf16, tag="e", name="e_t"
                            )
                            boundary = not (2 * u + 1 < 4 * g)
                            for a in range(2):
                                if not boundary:
                                    nc.scalar.activation(
                                        e_t[:, a],
                                        s_ps[a][:],
                                        AF.Exp,
                                        scale=float(D) ** -0.5,
                                    )
                                else:
                                    for jj in range(2):
                                        jt = 2 * u + jj
                                        cs = P * (jt - 4 * g)
                                        nc.scalar.activation(
                                            e_t[:, a, jj, cs:512],
                                            s_ps[a][:, jj, cs:512],
                                            AF.Exp,
                                            scale=float(D) ** -0.5,
                                        )
                            if boundary:
                                # one masked multiply covers both heads' diagonal
                                # blocks (tri broadcast along the head dim)
                                for jj in range(2):
                                    jt = 2 * u + jj
                                    cs = P * (jt - 4 * g)
                                    nc.vector.tensor_tensor(
                                        e_t[:, :, jj, cs : cs + P],
                                        e_t[:, :, jj, cs : cs + P],
                                        tri_sb.rearrange(
                                            "p (o c) -> p o c", o=1
                                        ).to_broadcast((P, 2, P)),
                                        OP.mult,
                                    )
                            for jj in range(2):
                                jt = 2 * u + jj
                                cs = P * (jt - 4 * g) if jt >= 4 * g else 0
                                for a in range(2):
                                    h = 2 * hp + a
                                    nc.tensor.matmul(
                                        o_ps[a][:, cs:512],
                                        v_sb[:, jt, (D + 1) * h : (D + 1) * (h + 1)],
                                        e_t[:, a, jj, cs:512],
                                        start=(jt == 0),
                                        stop=(jt == njt - 1),
                                    )
                            pop_filler()
                        # normalization for the pair: one fast SBUF copy of
                        # o_ps releases the PSUM accumulator's WAR (so the
                        # next head-pair's AV matmuls can start immediately);
                        # the reciprocal/broadcast/mult chain then runs from
                        # SBUF off the critical path
                        for a in range(2):
                            if COPIES_OFF or "norm_off" in ablate:
                                continue
                            rec = bc_p.tile([1, 512], f32, tag="rec", name="rec")
                            if "no_recip" in ablate:
                                nc.vector.memset(rec[:], 0.01)
                            else:
                                # ~5x faster than reciprocal(); sums >= 1 so no
                                # denorm/inf edge cases, 18 bits is plenty.  The
                                # bitwise exponent-flip seed needs an SBUF fp32
                                # source, so stage the sums row out of PSUM.
                                sums_sb = bc_p.tile(
                                    [1, 512], f32, tag="sums", name="sums_sb"
                                )
                                nc.vector.tensor_copy(
                                    sums_sb[:], o_ps[a][D : D + 1, :]
                                )
                                nc.vector.reciprocal_approx_fast(
                                    rec[:], sums_sb[:]
                                )
                            if "no_bcast" in ablate:
                                bc_sb = bc_fixed
                            else:
                                bc_sb = bc_p.tile(
                                    [D, 512], f32, tag="bc", name="bc_sb"
                                )
                                nc.gpsimd.partition_broadcast(bc_sb[:], rec[:])
                            if "no_mult" in ablate:
                                nc.vector.tensor_copy(
                                    onorm[
                                        64 * a : 64 * a + 64,
                                        hp,
                                        512 * g : 512 * (g + 1),
                                    ],
                                    bc_sb[:],
                                )
                            else:
                                nc.vector.tensor_tensor(
                                    onorm[
                                        64 * a : 64 * a + 64,
                                        hp,
                                        512 * g : 512 * (g + 1),
                                    ],
                                    o_ps[a][:D, :],
                                    bc_sb[:],
                                    OP.mult,
                                )

                    # out-proj chunks for this group become fillers, except the
                    # last group which must run now
                    new_chunks = []
                    for s in range(0, 4, 2):
                        ip = (4 * g + s) // 2
                        out_tiles[ip] = out_p.tile(
                            [P, 2, EMB], bf16, tag="osb", name="osb"
                        )
                    for s in range(4):
                        i = 4 * g + s
                        for n in range(2):
                            new_chunks.append(outproj_chunk(i, n))
                    if g == NG - 1:
                        for fn_ in defer_fillers:
                            fn_()
                        defer_fillers = []
                        for fn_ in new_chunks:
                            fn_()
                    else:
                        defer_fillers += new_chunks

    nc.compile()
    return nc


EK_ = EMB // P  # 8


def prep_in_maps(x, Wqkv, bqkv, Wout, bout):
    import ml_dtypes

    bf = ml_dtypes.bfloat16
    x = np.asarray(x, dtype=np.float32)
    Wqkv = np.asarray(Wqkv, dtype=np.float32)
    bqkv = np.asarray(bqkv, dtype=np.float32)
    Wout = np.asarray(Wout, dtype=np.float32)
    EK = EK_

    trimask = np.triu(np.ones((P, P), dtype=np.float32)).astype(bf)
    # x_pre[b][p, g, e, tl] = x[b].T[e*128+p, g*512+tl]
    x_pre = []
    for b in range(B):
        xbT = np.ascontiguousarray(x[b].T)  # [EMB, T]
        x_pre.append(
            np.ascontiguousarray(
                xbT.reshape(EK, P, NG, 512).transpose(1, 2, 0, 3)
            ).astype(bf).reshape(P, NG * EK * 512)
        )

    in_maps = []
    for c in range(N_CORES):
        b, m = divmod(c, TP)
        q0 = FSH * m
        wqk = np.concatenate(
            [Wqkv[:, q0 : q0 + FSH], Wqkv[:, H * D + q0 : H * D + q0 + FSH]], axis=1
        )  # [EMB, 512]
        wqk_pre = (
            np.ascontiguousarray(wqk.reshape(EK, P, 4, P).transpose(1, 2, 0, 3))
            .astype(bf)
            .reshape(P, 4 * EK * P)
        )
        wv = Wqkv[:, 2 * H * D + q0 : 2 * H * D + q0 + FSH]  # [EMB, 256]
        wv_pre = (
            np.ascontiguousarray(wv.reshape(EK, P, FSH).transpose(1, 0, 2))
            .astype(bf)
            .reshape(P, EK * FSH)
        )
        wout = Wout[q0 : q0 + FSH, :]  # [256, EMB]
        wout_pre = (
            np.ascontiguousarray(wout.reshape(2, P, EMB).transpose(1, 0, 2))
            .astype(bf)
            .reshape(P, 2 * EMB)
        )
        bqk = np.concatenate(
            [bqkv[q0 : q0 + FSH], bqkv[H * D + q0 : H * D + q0 + FSH]]
        )
        in_maps.append(
            {
                "xT": x_pre[b],
                "wqk": wqk_pre,
                "wv": wv_pre,
                "wout": wout_pre,
                "bqk": np.ascontiguousarray(bqk.reshape(4, P).T),
                "trimask": trimask,
            }
        )

    return in_maps


def assemble_output(results, inputs):
    bqkv = np.asarray(inputs["bqkv"], dtype=np.float32)
    Wout = np.asarray(inputs["Wout"], dtype=np.float32)
    bout = np.asarray(inputs["bout"], dtype=np.float32)
    # v-bias contribution folded into the output bias (attn rows sum to 1)
    bout_eff = bout + bqkv[2 * H * D :] @ Wout

    out = np.empty((B, T, EMB), dtype=np.float32)
    for b in range(B):
        acc = results[TP * b]["out"].astype(np.float32).copy()
        for m in range(1, TP):
            acc += results[TP * b + m]["out"]
        # acc layout [P, (NT//2) * 2 * EMB] -> [T, EMB]
        out[b] = (
            acc.reshape(P, NT, EMB).transpose(1, 0, 2).reshape(T, EMB) + bout_eff
        )
    return out


def kernel(x, Wqkv, bqkv, Wout, bout):
    from concourse.bass_utils import run_bass_kernel_spmd

    if "nc" not in _prog_cache:
        _prog_cache["nc"] = _build_program()
    nc = _prog_cache["nc"]

    in_maps = prep_in_maps(x, Wqkv, bqkv, Wout, bout)
    res = run_bass_kernel_spmd(nc, in_maps, core_ids=list(range(N_CORES)))
    _prog_cache["last_result"] = res

    inputs = {"x": x, "Wqkv": Wqkv, "bqkv": bqkv, "Wout": Wout, "bout": bout}
    return assemble_output(res.results, inputs)
